# revision 1
# baseline (speedup 1.0000x reference)
"""Trainium2 Bass kernel for nn_ContrastiveLoss (B=32,C=256,N=64,S=128,M=256).

Strategy (data-parallel over videos, 4 per core):
  Device (per core, per video b):
    - load video_feats[b] as two [128, 4096] fp32 c-halves
    - squares (ACT + GPSIMD) -> bf16, norm2 via stationary-sq matmuls
      -> invr = 1/max(sqrt(n2),1e-12) -> broadcast row to [128, 4096]
    - Q = sfT_n.T @ vf via f32r matmuls (+ rank-1 deadpen add that zeroes
      masked-out proposal columns after exp)
    - qs = (Q/T) * invr ; E = exp(qs) (bf16) with per-row accumulated sums
    - outputs: per-(b) row sums A (over live columns) and the E rows of the
      core's own sentences (for the host-side own-video mask correction)
  Host: sentence normalization/permutation, top-1 proposal selection, the
    tiny [M,S] logit matrix, loss assembly in float64.
"""

import os
import sys

sys.path.insert(0, "/opt/trn_rl_repo")

from contextlib import ExitStack

import ml_dtypes
import numpy as np

import concourse.bacc as bacc
import concourse.bass as bass
import concourse.mybir as mybir
import concourse.tile as tile
from concourse import library_config, masks
from concourse.bass_utils import run_bass_kernel_spmd

T_V, T_Q, NEG_IOU, K_TOP = 0.1, 0.1, 0.5, 1

N_CORES = 8
B, C, N = 32, 256, 64
NN = N * N  # 4096
S = 128
M = 256
BL = B // N_CORES  # videos per core
CH = C // 128  # c-halves
QCH = 1024  # Q free-dim chunk
QN = NN // QCH  # 4

F32 = mybir.dt.float32
F32R = mybir.dt.float32r
BF16 = mybir.dt.bfloat16
AO = mybir.AluOpType
AF = mybir.ActivationFunctionType

LAST_PROFILE = {}

_prog_cache = {}


def _build_program(W: int):
    """SPMD program for one core; W = padded own-sentence window rows."""
    nc = bacc.Bacc("TRN2", debug=False)

    vf_t = nc.dram_tensor("vf", [BL, C, NN], F32, kind="ExternalInput").ap()
    sfT_t = nc.dram_tensor("sfT", [CH, 128, S], F32, kind="ExternalInput").ap()
    lv_t = nc.dram_tensor("live32", [32, 128], F32, kind="ExternalInput").ap()
    a_t = nc.dram_tensor("a_out", [S, BL], F32, kind="ExternalOutput").ap()
    ew_t = nc.dram_tensor("e_win", [BL, W, NN], BF16, kind="ExternalOutput").ap()

    with tile.TileContext(nc) as tc, ExitStack() as ctx:
        consts = ctx.enter_context(tc.tile_pool(name="consts", bufs=1))
        vpool = ctx.enter_context(tc.tile_pool(name="v", bufs=3))
        sqpool = ctx.enter_context(tc.tile_pool(name="sq", bufs=1))
        ibpool = ctx.enter_context(tc.tile_pool(name="ib", bufs=2))
        npool = ctx.enter_context(tc.tile_pool(name="norm", bufs=2))
        qspool = ctx.enter_context(tc.tile_pool(name="qs", bufs=3))
        epool = ctx.enter_context(tc.tile_pool(name="e", bufs=2))
        apool = ctx.enter_context(tc.tile_pool(name="acc", bufs=1))
        psq = ctx.enter_context(tc.tile_pool(name="psq", bufs=2, space="PSUM"))
        psn = ctx.enter_context(tc.tile_pool(name="psn", bufs=2, space="PSUM"))
        pst = ctx.enter_context(tc.tile_pool(name="pst", bufs=2, space="PSUM"))

        nc.gpsimd.load_library(library_config.proxy)

        SFT = consts.tile([128, CH, S], F32R)
        for h in range(CH):
            nc.sync.dma_start(SFT[:, h, :], sfT_t[h].bitcast(F32R))
        LIVE = consts.tile([32, 128], F32)
        nc.sync.dma_start(LIVE[:], lv_t)
        ones_col = consts.tile([128, 1], BF16)
        nc.vector.memset(ones_col[:], 1.0)
        ident = consts.tile([128, 128], F32)
        masks.make_identity(nc, ident[:])
        aparts = apool.tile([S, BL * QN], F32)

        for b in range(BL):
            # ---- load video b (two c-halves, 4 DMAs for queue spread) ----
            V = vpool.tile([128, CH, NN], F32R)
            nc.sync.dma_start(
                V[:], vf_t[b].rearrange("(h p) j -> p h j", h=CH).bitcast(F32R)
            )

            # ---- squares (split engines) -> bf16 ----
            SQ = sqpool.tile([128, CH, NN], BF16)
            nc.scalar.square(SQ[:, 0, :], V[:, 0, :].bitcast(F32))
            nc.gpsimd.tensor_tensor(
                SQ[:, 1, :],
                V[:, 1, :].bitcast(F32),
                V[:, 1, :].bitcast(F32),
                op=AO.mult,
            )

            # ---- norm2: stationary-sq matmuls -> n2p[p, n] = n2[n*128+p] ----
            n2p = psn.tile([128, 32], F32)
            for n in range(32):
                sl = slice(n * 128, (n + 1) * 128)
                nc.tensor.matmul(
                    n2p[:, n : n + 1], SQ[:, 0, sl], ones_col[:], start=True, stop=False
                )
                nc.tensor.matmul(
                    n2p[:, n : n + 1], SQ[:, 1, sl], ones_col[:], start=False, stop=True
                )
            N2S = npool.tile([128, 32], F32, tag="n2s")
            nc.vector.tensor_copy(N2S[:], n2p[:])

            # ---- invr: transpose -> [32, 128], max/sqrt/recip ----
            N2TP = pst.tile([32, 128], F32)
            nc.tensor.transpose(N2TP[:], N2S[:], ident[:])
            T_NM = npool.tile([32, 128], F32, tag="nm")
            nc.vector.tensor_scalar_max(T_NM[:], N2TP[:], 1e-24)
            T_RT = npool.tile([32, 128], F32, tag="rt")
            nc.scalar.sqrt(T_RT[:], T_NM[:])
            T_IVR = npool.tile([32, 128], F32, tag="ivr")
            nc.vector.reciprocal(T_IVR[:], T_RT[:])
            T_IV = npool.tile([32, 128], F32, tag="iv")
            nc.vector.tensor_mul(T_IV[:], T_IVR[:], LIVE[:])
            # ---- broadcast invr -> [128, NN]: reshape into row 0, pbcast ----
            IB = ibpool.tile([128, NN], F32)
            nc.scalar.dma_start(
                IB[0:1, :].rearrange("a (n q) -> a n q", n=32), T_IV[:]
            )
            nc.gpsimd.partition_broadcast(IB[:], IB[0:1, :])

            # ---- Q chunks: matmul f32r + deadpen, scale, exp, ship ----
            E = epool.tile([128, NN], BF16)
            for n in range(QN):
                q = psq.tile([128, QCH], F32)
                for g in range(QCH // 512):
                    lo = n * QCH + g * 512
                    sl = slice(lo, lo + 512)
                    gl = slice(g * 512, (g + 1) * 512)
                    nc.tensor.matmul(
                        q[:, gl], SFT[:, 0, :], V[:, 0, sl], start=True, stop=False
                    )
                    nc.tensor.matmul(
                        q[:, gl], SFT[:, 1, :], V[:, 1, sl], start=False, stop=True
                    )
                qs = qspool.tile([128, QCH], F32)
                nc.vector.scalar_tensor_tensor(
                    qs[:],
                    q[:],
                    1.0 / T_Q,
                    IB[:, n * QCH : (n + 1) * QCH],
                    op0=AO.mult,
                    op1=AO.mult,
                )
                nc.scalar.activation(
                    E[:, n * QCH : (n + 1) * QCH],
                    qs[:],
                    AF.Exp,
                    accum_out=aparts[:, b * QN + n : b * QN + n + 1],
                )

            nc.scalar.dma_start(ew_t[b], E[0:W, :])

        # ---- A reduce: a_out[s, b] = sum_n aparts[s, b*QN+n] ----
        AOT = consts.tile([S, BL], F32)
        for b in range(BL):
            nc.vector.reduce_sum(
                AOT[:, b : b + 1],
                aparts[:, b * QN : (b + 1) * QN],
                axis=mybir.AxisListType.X,
            )
        nc.sync.dma_start(a_t, AOT[:])

    nc.compile()
    return nc


def _host_reference(video_feats, sents_feats, iou2d, iou2ds, mask_idx, s2v, m2s):
    """Pure-numpy fallback (used only for degenerate inputs, e.g. duplicate
    mask indices or a batch that does not divide across cores)."""
    vf = video_feats.reshape(video_feats.shape[0], video_feats.shape[1], -1)
    Bx, Cx, NNx = vf.shape
    Sx = sents_feats.shape[0]
    Mx = iou2ds.shape[0]
    vfm = vf[:, :, mask_idx].transpose(0, 2, 1).astype(np.float64)  # [B,P,C]
    vfn = vfm / np.maximum(np.linalg.norm(vfm, axis=-1, keepdims=True), 1e-12)
    sfn = sents_feats.astype(np.float64)
    sfn = sfn / np.maximum(np.linalg.norm(sfn, axis=-1, keepdims=True), 1e-12)
    iou2d_f = iou2d.reshape(Sx, -1)[:, mask_idx]
    iou2ds_f = iou2ds.reshape(Mx, -1)[:, mask_idx]
    m2v = s2v[m2s]
    topk = np.argmax(iou2ds_f, axis=1)
    topv = vfn[m2v, topk]  # [M, C]
    pos = (topv * sfn[m2s]).sum(-1)
    allv = topv @ sfn.T  # [M, S]
    e1 = np.exp(allv / T_V)
    neg1 = e1.sum(1) - e1[np.arange(Mx), m2s]
    loss1 = np.mean(-(pos / T_V - np.log(np.exp(pos / T_V) + neg1)))
    all_q = np.einsum("sc,bpc->sbp", sfn, vfn)  # [S,B,P]
    pos_mask = np.zeros(all_q.shape, dtype=bool)
    pos_mask[np.arange(Sx), s2v] = iou2d_f > NEG_IOU
    e2 = np.exp(all_q / T_Q) * ~pos_mask
    neg2 = e2.reshape(Sx, -1).sum(1)
    loss2 = np.mean(-(pos / T_Q - np.log(np.exp(pos / T_Q) + neg2[m2s])))
    return np.float32(loss1), np.float32(loss2), np.float32(0.0)


def kernel(video_feats, sents_feats, iou2d, iou2ds, mask_idx, scatter_s2v, scatter_m2s):
    video_feats = np.ascontiguousarray(video_feats, dtype=np.float32)
    sents_feats = np.ascontiguousarray(sents_feats, dtype=np.float32)
    iou2d = np.ascontiguousarray(iou2d, dtype=np.float32)
    iou2ds = np.ascontiguousarray(iou2ds, dtype=np.float32)
    mask_idx = np.asarray(mask_idx)
    s2v = np.asarray(scatter_s2v).astype(np.int64)
    m2s = np.asarray(scatter_m2s).astype(np.int64)

    w = np.bincount(mask_idx.astype(np.int64), minlength=NN).astype(np.float64)
    if (
        video_feats.shape != (B, C, N, N)
        or sents_feats.shape != (S, C)
        or (w > 1).any()
        or s2v.min() < 0
        or s2v.max() >= B
        or m2s.min() < 0
        or m2s.max() >= S
    ):
        return _host_reference(
            video_feats, sents_feats, iou2d, iou2ds, mask_idx, s2v, m2s
        )

    vf = video_feats.reshape(B, C, NN)
    live = w > 0
    live32 = np.ascontiguousarray(
        live.astype(np.float32).reshape(32, 128)
    )  # j = n*128 + q layout matches T_IV
    n_dead = float((~live).sum())

    sfn = sents_feats.astype(np.float64)
    sfn = sfn / np.maximum(np.linalg.norm(sfn, axis=-1, keepdims=True), 1e-12)
    sfn32 = sfn.astype(np.float32)

    m2v = s2v[m2s]

    # top-1 proposal per moment (host), with exact reference tie-breaking
    iou2ds_f = iou2ds.reshape(M, NN)[:, mask_idx]
    topk_j = mask_idx[np.argmax(iou2ds_f, axis=1)]

    # gathered top proposal features + their norms (host fp64)
    tv = vf[m2v, :, topk_j].astype(np.float64)  # [M, C]
    r_m = np.maximum(np.sqrt((tv**2).sum(-1)), 1e-12)
    out1 = (tv / r_m[:, None]) @ sfn.T  # [M, S]
    pos = out1[np.arange(M), m2s]

    # loss 1 entirely host
    e1 = np.exp(out1 / T_V)
    neg1 = e1.sum(1) - e1[np.arange(M), m2s]
    loss1 = np.mean(-(pos / T_V - np.log(np.exp(pos / T_V) + neg1)))

    # per-core metadata
    perms, owns = [], []
    W = 1
    for k in range(N_CORES):
        own = np.where((s2v >= k * BL) & (s2v < (k + 1) * BL))[0]
        rest = np.setdiff1d(np.arange(S), own)
        perms.append(np.concatenate([own, rest]))
        owns.append(own)
        W = max(W, len(own))

    key = W
    if key not in _prog_cache:
        _prog_cache[key] = _build_program(W)
    nc = _prog_cache[key]

    in_maps = []
    for k in range(N_CORES):
        perm = perms[k]
        sfT = np.ascontiguousarray(
            sfn32[perm].T.reshape(CH, 128, S)
        )  # [h, c_in_half, s_perm]
        in_maps.append(
            {
                "vf": np.ascontiguousarray(vf[k * BL : (k + 1) * BL]),
                "sfT": sfT,
                "live32": live32,
            }
        )

    res = run_bass_kernel_spmd(nc, in_maps, core_ids=list(range(N_CORES)))
    if res.exec_time_ns is not None:
        LAST_PROFILE["exec_time_ns"] = res.exec_time_ns

    # host combine
    iou_flat = iou2d.reshape(S, NN)
    A_total = np.zeros(S, dtype=np.float64)
    corr = np.zeros(S, dtype=np.float64)
    for k in range(N_CORES):
        r = res.results[k]
        a = np.asarray(r["a_out"], dtype=np.float64)  # [S(perm), BL]
        ew = np.asarray(r["e_win"], dtype=np.float64)  # [BL, W, NN]
        perm, own = perms[k], owns[k]
        A_total[perm] += a.sum(1) - BL * n_dead
        for i, s in enumerate(own):
            b_loc = int(s2v[s]) - k * BL
            row = ew[b_loc, i]
            corr[s] += row[live & (iou_flat[s] > NEG_IOU)].sum()

    negsum_q = A_total - corr
    loss2 = np.mean(-(pos / T_Q - np.log(np.exp(pos / T_Q) + negsum_q[m2s])))

    return np.float32(loss1), np.float32(loss2), np.float32(0.0)

